# revision 48
# baseline (speedup 1.0000x reference)
"""Trainium2 Bass kernel for the BayesianLayer problem.

Computes, for
    sigma   = softplus(ro)                      (IN, OUT)
    weights = eps * sigma + mu                  (B, R, IN, OUT)
    bias    = eps_bias * softplus(ro_bias) + mu_bias
    out     = mean_r(x @ weights + bias)        (B, OUT)
    log_prior = sum(log(mix(weights))) + sum(log(mix(bias)))   (scaled 1/(B*R))
    log_p     = sum(log(N(w; mu, sigma))) + bias-term          (scaled 1/(B*R))

Strategy: data-parallel over batch B across 8 NeuronCores (4 batches each).
eps is uploaded in bf16 (halves HBM traffic; elementwise math runs in DVE
2x/4x bf16 modes).  Key algebraic restructurings (all exact up to tiny,
quantified approximations):

  log(mix(w)) with the reference's clamps is exactly
      max( softplus(ln10 - 49.5*u) + softplus(u/2 - 22.1069) - u/2 - ln(2*sqrt(2pi)),
           ln(1e-10) )                      with u = w^2
  The second softplus (active only for |w| in ~[5.7, 6.8]) is dropped and the
  outer max is folded into a min-clamp on t0 = u/2, so per element we need
  only:  u = w^2 ; t0 = min(u/2, 21.41377) ; s1 = ln(1 + exp(-99*t0 + ln10));
  lm = s1 - t0 - ln(2*sqrt(2pi)).  The constant is folded on the host.

  log(N(w; mu, sigma)) = clamp(-eps^2/2 - log(sqrt(2pi)*sigma), ln 1e-10, ln 10)
  whose sum is  16*sum(C) - sum(eps^2/2)  per core (clamp corrections are
  O(1e-5) relative and neglected; C = -log(sqrt(2pi)*sigma)).

  einsum('bi,brio->bro') = x @ (eps*sigma)  [bf16 on PE]  +  x @ mu  [fp32 on PE].

The bias path (tiny: 16x1024 per core) is computed with the full exact
formulas in fp32, including both softplus terms and both clamps.

Self-contained: hardcodes all shapes; only needs concourse (+numpy/ml_dtypes).
"""

import math
import os
import sys
from contextlib import ExitStack

import numpy as np

for _p in ("/opt/trn_rl_repo",):
    if _p not in sys.path:
        sys.path.insert(0, _p)

import ml_dtypes  # noqa: E402

import concourse.bacc as bacc  # noqa: E402
import concourse.bass as bass  # noqa: E402
import concourse.mybir as mybir  # noqa: E402
import concourse.tile as tile  # noqa: E402
from concourse.bass_utils import run_bass_kernel_spmd  # noqa: E402

# ----------------------------------------------------------------------------
# Problem constants (hardcoded per the harness contract)
N_CORES = 8
B, R, IN, OUT = 32, 4, 1024, 1024
BPC = B // N_CORES            # batches per core = 4
PLANES = BPC * R              # eps planes per core = 16
NCHUNK = IN // 128            # 8 partition chunks of the contraction dim
NTILES = PLANES * NCHUNK      # 128 main-loop tiles per core
NW_CORE = PLANES * IN * OUT   # eps elements per core

LN_SQRT_2PI = 0.5 * math.log(2.0 * math.pi)          # 0.918938533
K2 = math.log(2.0) + LN_SQRT_2PI                     # ln(2*sqrt(2pi)) = 1.61208571
LNC = math.log(1e-10)                                # -23.02585093
LN10 = math.log(10.0)                                # 2.30258509
CLAMP0 = -LNC - K2                                   # 21.41376521
SP2_BIAS = LNC + LN_SQRT_2PI                         # -22.10691240

F32 = mybir.dt.float32
BF16 = mybir.dt.bfloat16
AF = mybir.ActivationFunctionType
ALU = mybir.AluOpType

# r-planes are processed in pairs: tiles are [128, 2*OUT] to amortize the
# per-instruction fixed costs (ACT: 352 cyc, DVE: 58 cyc) over more elements.
RPAIR = 2
NPAIRS = NTILES // RPAIR      # 64 paired tiles
# Every (H2_DVE_EVERY)-th paired tile computes sum(eps^2) on the DVE instead
# of the ACT engine, to balance the two engines' load.
H2_DVE_EVERY = 2

_CACHED = {}


def _pin_act_tables():
    """bacc's table chooser alternates Exp/Ln between two different table
    sets, paying a ~1.4us ACT_TABLE_LOAD per switch (240 loads!).  All the
    functions this kernel uses live together in natural_log_exp_and_others;
    strip them from every other set so the chooser has one stable answer."""
    if getattr(bacc, "_act_tables_pinned", False):
        return
    orig = bacc.get_activation_tables
    mine = {AF.Exp, AF.Ln, AF.Square, AF.Copy, AF.Identity}

    def patched(arch):
        tabs = orig(arch)
        return {
            name: (set(fns) if name == "natural_log_exp_and_others"
                   else set(fns) - mine)
            for name, fns in tabs.items()
        }

    bacc.get_activation_tables = patched
    bacc._act_tables_pinned = True


def _build_kernel():
    _pin_act_tables()
    nc = bacc.Bacc()

    eps_bf = nc.dram_tensor("eps_bf", [PLANES, IN, OUT], BF16, kind="ExternalInput")
    x_bf = nc.dram_tensor("x_bf", [128, NCHUNK * BPC], BF16, kind="ExternalInput")
    x_f32 = nc.dram_tensor("x_f32", [128, NCHUNK * BPC], F32, kind="ExternalInput")
    mu_d = nc.dram_tensor("mu", [IN, OUT], F32, kind="ExternalInput")
    ro_d = nc.dram_tensor("ro", [IN, OUT], F32, kind="ExternalInput")
    mu_b_d = nc.dram_tensor("mu_bias", [1, OUT], F32, kind="ExternalInput")
    ro_b_d = nc.dram_tensor("ro_bias", [1, OUT], F32, kind="ExternalInput")
    epsb_d = nc.dram_tensor("eps_bias", [PLANES, OUT], F32, kind="ExternalInput")
    blk_d = nc.dram_tensor("blk", [16, BPC], F32, kind="ExternalInput")

    out_d = nc.dram_tensor("out_shard", [BPC, OUT], F32, kind="ExternalOutput")
    part_d = nc.dram_tensor("partials", [1, 8], F32, kind="ExternalOutput")

    def bcast16(dram_h):
        ap0 = dram_h[:, :]
        return bass.AP(tensor=ap0.tensor, offset=ap0.offset, ap=[[0, 16], ap0.ap[-1]])

    with tile.TileContext(nc) as tc, ExitStack() as ctx:
        consts = ctx.enter_context(tc.tile_pool(name="consts", bufs=1))
        work = ctx.enter_context(tc.tile_pool(name="work", bufs=3))
        drain = ctx.enter_context(tc.tile_pool(name="drain", bufs=1))
        pre = ctx.enter_context(tc.tile_pool(name="pre", bufs=2))
        bias_p = ctx.enter_context(tc.tile_pool(name="biasp", bufs=1))
        ps_pre = ctx.enter_context(tc.tile_pool(name="ps_pre", bufs=1, space="PSUM"))
        ps_eins = ctx.enter_context(tc.tile_pool(name="ps_eins", bufs=1, space="PSUM"))
        ps_fin = ctx.enter_context(tc.tile_pool(name="ps_fin", bufs=1, space="PSUM"))

        # ------------------------- resident constants -------------------------
        sig_bf = consts.tile([128, NCHUNK, OUT], BF16)
        mu_bf = consts.tile([128, NCHUNK, OUT], BF16)
        x_bf_sb = consts.tile([128, NCHUNK * BPC], BF16)
        x_f32_sb = consts.tile([128, NCHUNK * BPC], F32)
        ones128 = consts.tile([128, 1], F32)
        blk_sb = consts.tile([16, BPC], F32)
        lm_slots = consts.tile([128, NPAIRS], F32)
        h2_slots = consts.tile([128, NPAIRS], F32)
        lnsig_slots = consts.tile([128, NCHUNK], F32)
        redbuf = consts.tile([128, 3], F32)
        redb16 = consts.tile([16, 2], F32)
        sum2_sb = consts.tile([BPC, OUT], F32)
        sum2_flat = consts.tile([1, BPC * OUT], F32)
        cb_ln10 = consts.tile([128, 1], F32)
        cb_sp2 = consts.tile([128, 1], F32)
        cb_a = consts.tile([128, 1], F32)

        A_K2 = math.exp(-K2)  # ln(A*x + A) = ln(x+1) - K2
        nc.vector.memset(cb_ln10, LN10)
        nc.vector.memset(cb_sp2, SP2_BIAS)
        nc.vector.memset(cb_a, A_K2)
        nc.vector.memset(ones128, 1.0)
        nc.gpsimd.dma_start(out=x_bf_sb, in_=x_bf[:, :])
        nc.gpsimd.dma_start(out=x_f32_sb, in_=x_f32[:, :])
        nc.gpsimd.dma_start(out=blk_sb, in_=blk_d[:, :])

        # --------- preamble: sigma = softplus(ro) (bf16), mu cast, x@mu -------
        # sig_bf/mu_bf hold sigma/sqrt(2) and mu/sqrt(2) so that the squared
        # pre-activation w'^2 equals w^2/2 directly (saves a DVE op per tile).
        INV_SQRT2 = 0.70710678118654752
        xmu_ps = ps_pre.tile([BPC, OUT], F32)
        # dummy matmul: PE instructions may carry only one sync wait, so make
        # PE "see" x_f32_sb here; the real group start=True resets the region.
        nc.tensor.matmul(
            xmu_ps[0:BPC, 0:BPC], x_f32_sb[:, 0:BPC], x_f32_sb[:, 0:BPC],
            start=True, stop=True,
        )
        for c in range(NCHUNK):
            ro_t = pre.tile([128, OUT], F32)
            nc.gpsimd.dma_start(out=ro_t, in_=ro_d[c * 128:(c + 1) * 128, :])
            nc.scalar.activation(ro_t, ro_t, AF.Exp)
            sig_f = pre.tile([128, OUT], F32)
            nc.scalar.activation(sig_f, ro_t, AF.Ln, bias=1.0)  # sigma fp32
            lnsig_scr = pre.tile([128, OUT], BF16)
            nc.scalar.activation(
                lnsig_scr, sig_f, AF.Ln,
                accum_out=lnsig_slots[:, c:c + 1],
            )
            nc.scalar.activation(sig_bf[:, c, :], sig_f, AF.Copy, scale=INV_SQRT2)
            mu_t = pre.tile([128, OUT], F32)
            nc.gpsimd.dma_start(out=mu_t, in_=mu_d[c * 128:(c + 1) * 128, :])
            nc.vector.tensor_scalar(
                out=mu_bf[:, c, :], in0=mu_t, scalar1=INV_SQRT2, scalar2=None,
                op0=ALU.mult,
            )
            for h in range(2):
                nc.tensor.matmul(
                    xmu_ps[0:BPC, h * 512:(h + 1) * 512],
                    x_f32_sb[:, c * BPC:(c + 1) * BPC],
                    mu_t[:, h * 512:(h + 1) * 512],
                    start=(c == 0),
                    stop=(c == NCHUNK - 1),
                )

        # drain x@mu now so its PSUM slot can be reused by the bias-sum matmul
        xmub_sb = bias_p.tile([BPC, OUT], F32)
        nc.scalar.activation(xmub_sb, xmu_ps, AF.Copy)

        # ----------------------------- bias path ------------------------------
        # everything fp32, exact formulas (with both softplus terms and clamps);
        # tiles are aggressively reused in place (tiny path, off the hot loop).
        eb = bias_p.tile([16, OUT], F32)
        nc.gpsimd.dma_start(out=eb, in_=epsb_d[:, :])
        sig_b = bias_p.tile([16, OUT], F32)
        nc.gpsimd.dma_start(out=sig_b, in_=bcast16(ro_b_d))
        mu_b16 = bias_p.tile([16, OUT], F32)
        nc.gpsimd.dma_start(out=mu_b16, in_=bcast16(mu_b_d))

        nc.scalar.activation(sig_b, sig_b, AF.Exp)
        nc.scalar.activation(sig_b, sig_b, AF.Ln, bias=1.0)  # sigma_b
        c_b = bias_p.tile([16, OUT], F32)
        nc.scalar.activation(c_b, sig_b, AF.Ln)
        nc.vector.tensor_scalar(
            out=c_b, in0=c_b, scalar1=-1.0, scalar2=-LN_SQRT_2PI,
            op0=ALU.mult, op1=ALU.add,
        )
        # gaussian log-term: clamp(C_b - eb^2/2, LNC, LN10), exact, accumulated
        h2b = bias_p.tile([16, OUT], F32)
        nc.scalar.activation(h2b, eb, AF.Square, scale=0.70710678118654752)
        nc.vector.tensor_tensor(out=h2b, in0=c_b, in1=h2b, op=ALU.subtract)
        nc.vector.tensor_scalar(
            out=h2b, in0=h2b, scalar1=LN10, scalar2=LNC,
            op0=ALU.min, op1=ALU.max,
        )
        nc.vector.tensor_scalar(
            out=h2b, in0=h2b, scalar1=1.0, scalar2=0.0,
            op0=ALU.mult, op1=ALU.add, accum_out=redb16[:, 1:2],
        )
        # mixture log-term, exact with both softplus terms and final max-clamp
        w_b = bias_p.tile([16, OUT], F32)
        nc.vector.tensor_tensor(out=w_b, in0=eb, in1=sig_b, op=ALU.mult)
        nc.vector.tensor_tensor(out=w_b, in0=w_b, in1=mu_b16, op=ALU.add)
        u_b = bias_p.tile([16, OUT], F32)
        nc.vector.tensor_tensor(out=u_b, in0=w_b, in1=w_b, op=ALU.mult)
        t0u_b = bias_p.tile([16, OUT], F32)
        nc.vector.tensor_scalar(
            out=t0u_b, in0=u_b, scalar1=0.5, scalar2=None, op0=ALU.mult,
        )
        s1_b = bias_p.tile([16, OUT], F32)
        nc.scalar.activation(s1_b, t0u_b, AF.Exp, scale=-99.0, bias=cb_ln10[0:16, :])
        nc.scalar.activation(s1_b, s1_b, AF.Ln, bias=1.0)
        # u_b tile becomes t0s -> rho2 -> s2 in place
        nc.vector.tensor_scalar(
            out=u_b, in0=u_b, scalar1=0.5, scalar2=30.0,
            op0=ALU.mult, op1=ALU.min,
        )
        nc.scalar.activation(u_b, u_b, AF.Exp, scale=1.0, bias=cb_sp2[0:16, :])
        nc.scalar.activation(u_b, u_b, AF.Ln, bias=1.0)  # s2_b
        tmp_b = bias_p.tile([16, OUT], F32)
        nc.vector.scalar_tensor_tensor(
            out=tmp_b, in0=t0u_b, scalar=-1.0, in1=s1_b,
            op0=ALU.mult, op1=ALU.add,
        )
        nc.vector.tensor_tensor(out=tmp_b, in0=tmp_b, in1=u_b, op=ALU.add)
        nc.vector.tensor_scalar(
            out=tmp_b, in0=tmp_b, scalar1=-K2, scalar2=LNC,
            op0=ALU.add, op1=ALU.max,
        )
        nc.vector.tensor_scalar(
            out=tmp_b, in0=tmp_b, scalar1=1.0, scalar2=0.0,
            op0=ALU.mult, op1=ALU.add, accum_out=redb16[:, 0:1],
        )
        # per-b sums of bias values (for the output):  blk.T @ w_b
        bias_ps = ps_pre.tile([BPC, OUT], F32)
        for h in range(2):
            nc.tensor.matmul(
                bias_ps[0:BPC, h * 512:(h + 1) * 512],
                blk_sb[:, 0:BPC],
                w_b[:, h * 512:(h + 1) * 512],
                start=True, stop=True,
            )
        # sum2 = x@mu + 0.25 * bias_sums
        nc.scalar.activation(sum2_sb, bias_ps, AF.Copy, scale=0.25)
        nc.vector.tensor_tensor(out=sum2_sb, in0=xmub_sb, in1=sum2_sb, op=ALU.add)
        # engines can't address partition offsets 1-3; flatten rows onto
        # partition 0 with an SBUF->SBUF DMA so the per-b drain can read them
        nc.gpsimd.dma_start(
            out=sum2_flat[0:1, :].rearrange("o (b f) -> o b f", b=BPC),
            in_=sum2_sb[:, :],
        )

        # ------------------------------ main loop ------------------------------
        # One pair of PSUM banks holds all four b-outputs: batch b accumulates
        # into partition row 32*b via PE column-group tiling.  No PSUM slot
        # reuse -> each group's first matmul carries only its single w1 wait.
        eins_lo = ps_eins.tile([128, 512], F32)
        eins_hi = ps_eins.tile([128, 512], F32)
        eins_h = [eins_lo, eins_hi]
        nc.tensor.matmul(
            eins_h[0][0:1, 0:4], x_bf_sb[:, 0:1], x_bf_sb[:, 0:4],
            start=True, stop=True,
        )
        def rep2(ap2d):
            # [128, N] operand viewed as [128, 2, N] via a stride-0 middle dim
            return bass.AP(
                tensor=ap2d.tensor, offset=ap2d.offset,
                ap=[ap2d.ap[0], [0, RPAIR], ap2d.ap[-1]],
            )

        for b in range(BPC):
            for rp in range(R // RPAIR):
                for c in range(NCHUNK):
                    k = (b * (R // RPAIR) + rp) * NCHUNK + c
                    t_e = work.tile([128, RPAIR, OUT], BF16)
                    for ri in range(RPAIR):
                        plane = b * R + rp * RPAIR + ri
                        nc.gpsimd.dma_start(
                            out=t_e[:, ri, :],
                            in_=eps_bf[plane, c * 128:(c + 1) * 128, :],
                        )
                    sig_c = rep2(sig_bf[:, c, :])
                    mu_c = rep2(mu_bf[:, c, :])
                    w1 = work.tile([128, RPAIR, OUT], BF16)
                    nc.vector.tensor_tensor(out=w1, in0=t_e, in1=sig_c, op=ALU.mult)
                    # wt holds w' = w/sqrt(2), then t0 = w'^2 = w^2/2 (in place)
                    wt = work.tile([128, RPAIR, OUT], BF16)
                    nc.vector.tensor_tensor(out=wt, in0=w1, in1=mu_c, op=ALU.add)
                    nc.vector.tensor_tensor(out=wt, in0=wt, in1=wt, op=ALU.mult)
                    # rho -> s1-K2 -> y = s1-K2-t0 (in place); the -K2 constant
                    # rides the Ln as ln(A*rho + A) = ln(rho+1) - K2
                    rho = work.tile([128, RPAIR, OUT], BF16)
                    nc.scalar.activation(rho, wt, AF.Exp, scale=-99.0, bias=cb_ln10)
                    nc.scalar.activation(rho, rho, AF.Ln, scale=A_K2, bias=cb_a)
                    nc.vector.tensor_tensor(out=rho, in0=rho, in1=wt, op=ALU.subtract)
                    # lm = max(y, LNC) elementwise (op0); op1 is the reduction
                    scr = work.tile([128, RPAIR, OUT], BF16)
                    nc.vector.tensor_scalar(
                        out=scr, in0=rho, scalar1=LNC, scalar2=0.0,
                        op0=ALU.max, op1=ALU.add,
                        accum_out=lm_slots[:, k:k + 1],
                    )
                    scr2 = work.tile([128, RPAIR, OUT], BF16)
                    if k % H2_DVE_EVERY == H2_DVE_EVERY - 1:
                        nc.vector.tensor_tensor(
                            out=scr2, in0=t_e, in1=t_e, op=ALU.mult
                        )
                        nc.vector.tensor_scalar(
                            out=scr2, in0=scr2, scalar1=1.0, scalar2=0.0,
                            op0=ALU.mult, op1=ALU.add,
                            accum_out=h2_slots[:, k:k + 1],
                        )
                    else:
                        nc.scalar.activation(
                            scr2, t_e, AF.Square, scale=1.0,
                            accum_out=h2_slots[:, k:k + 1],
                        )
                    for ri in range(RPAIR):
                        r = rp * RPAIR + ri
                        for h in range(2):
                            nc.tensor.matmul(
                                eins_h[h][32 * b:32 * b + 1, :],
                                x_bf_sb[:, c * BPC + b:c * BPC + b + 1],
                                w1[:, ri, h * 512:(h + 1) * 512],
                                start=(r == 0 and c == 0),
                                stop=(r == R - 1 and c == NCHUNK - 1),
                                tile_position=(0, 32 * b),
                            )

        # drain: out_row = sqrt(2)/4 * einsum + (x@mu + bias_sums/4)
        for b in range(BPC):
            e_sb = drain.tile([1, OUT], F32)
            for h in range(2):
                nc.vector.tensor_scalar(
                    out=e_sb[:, h * 512:(h + 1) * 512],
                    in0=eins_h[h][32 * b:32 * b + 1, :],
                    scalar1=0.35355339059327373, scalar2=None, op0=ALU.mult,
                )
            out_row = drain.tile([1, OUT], F32)
            nc.vector.tensor_tensor(
                out=out_row, in0=e_sb,
                in1=sum2_flat[0:1, b * OUT:(b + 1) * OUT], op=ALU.add
            )
            nc.gpsimd.dma_start(out=out_d[b:b + 1, :], in_=out_row)

        # ------------------------------- finale --------------------------------
        nc.vector.tensor_reduce(
            out=redbuf[:, 0:1], in_=lm_slots, axis=mybir.AxisListType.X, op=ALU.add
        )
        nc.vector.tensor_reduce(
            out=redbuf[:, 1:2], in_=h2_slots, axis=mybir.AxisListType.X, op=ALU.add
        )
        nc.vector.tensor_reduce(
            out=redbuf[:, 2:3], in_=lnsig_slots, axis=mybir.AxisListType.X, op=ALU.add
        )
        fin_ps = ps_fin.tile([1, 8], F32)
        nc.tensor.matmul(
            fin_ps[0:1, 0:3], ones128[:, 0:1], redbuf[:, 0:3], start=True, stop=True
        )
        nc.tensor.matmul(
            fin_ps[0:1, 3:5], ones128[0:16, 0:1], redb16[:, 0:2],
            start=True, stop=True,
        )
        fin_sb = consts.tile([1, 8], F32)
        nc.vector.memset(fin_sb, 0.0)
        nc.scalar.activation(fin_sb[0:1, 0:5], fin_ps[0:1, 0:5], AF.Copy)
        nc.gpsimd.dma_start(out=part_d[:, :], in_=fin_sb)

    nc.compile()
    return nc


def _prepare_in_maps(x, mu, ro, mu_bias, ro_bias, eps, eps_bias):
    x = np.asarray(x, np.float32)
    mu = np.asarray(mu, np.float32)
    ro = np.asarray(ro, np.float32)
    mu_bias = np.asarray(mu_bias, np.float32).reshape(1, OUT)
    ro_bias = np.asarray(ro_bias, np.float32).reshape(1, OUT)
    eps = np.asarray(eps)
    eps_bias = np.asarray(eps_bias, np.float32)

    blk = np.zeros((16, BPC), np.float32)
    for p in range(16):
        blk[p, p // R] = 1.0

    in_maps = []
    for core in range(N_CORES):
        b0 = core * BPC
        eps_sh = np.ascontiguousarray(eps[b0:b0 + BPC]).reshape(PLANES, IN, OUT)
        eps_sh = eps_sh.astype(ml_dtypes.bfloat16)
        x_sh = x[b0:b0 + BPC]                      # [BPC, IN]
        # x_cols[p, c*BPC + b] = x[b, c*128 + p]
        x_cols = np.ascontiguousarray(
            x_sh.reshape(BPC, NCHUNK, 128).transpose(2, 1, 0).reshape(128, NCHUNK * BPC)
        ).astype(np.float32)
        in_maps.append({
            "eps_bf": eps_sh,
            "x_bf": x_cols.astype(ml_dtypes.bfloat16),
            "x_f32": x_cols,
            "mu": mu,
            "ro": ro,
            "mu_bias": mu_bias,
            "ro_bias": ro_bias,
            "eps_bias": np.ascontiguousarray(
                eps_bias[b0:b0 + BPC]).reshape(PLANES, OUT),
            "blk": blk,
        })
    return in_maps


LAST_EXEC_NS = None


def kernel(x, mu, ro, mu_bias, ro_bias, eps, eps_bias):
    global LAST_EXEC_NS
    if "nc" not in _CACHED:
        _CACHED["nc"] = _build_kernel()
    nc = _CACHED["nc"]

    in_maps = _prepare_in_maps(x, mu, ro, mu_bias, ro_bias, eps, eps_bias)

    trace = bool(int(os.environ.get("KERNEL_TRACE", "0")))
    res = run_bass_kernel_spmd(
        nc, in_maps, list(range(N_CORES)), trace=trace,
    )
    LAST_EXEC_NS = res.exec_time_ns
    results = res.results

    out = np.concatenate(
        [np.asarray(results[c]["out_shard"], np.float64) for c in range(N_CORES)],
        axis=0,
    )

    denom = float(B * R)
    lp_sum = 0.0
    lpw_sum = 0.0
    for c in range(N_CORES):
        p = np.asarray(results[c]["partials"], np.float64).reshape(-1)
        lm_w, lm_b, sum_eps2, lnsig, lg_b = p[0], p[3], p[1], p[2], p[4]
        sum_c = -(IN * OUT) * LN_SQRT_2PI - lnsig
        lg_w = PLANES * sum_c - 0.5 * sum_eps2
        lp_sum += lm_w + lm_b
        lpw_sum += lg_w + lg_b
    log_prior = np.float32(lp_sum / denom)
    log_p = np.float32(lpw_sum / denom)

    return out.astype(np.float32), log_prior, log_p


# revision 64
# speedup vs baseline: 1.3905x; 1.3905x over previous
"""Trainium2 Bass kernel for the BayesianLayer problem.

Computes, for
    sigma   = softplus(ro)                      (IN, OUT)
    weights = eps * sigma + mu                  (B, R, IN, OUT)
    bias    = eps_bias * softplus(ro_bias) + mu_bias
    out     = mean_r(x @ weights + bias)        (B, OUT)
    log_prior = sum(log(mix(weights))) + sum(log(mix(bias)))   (scaled 1/(B*R))
    log_p     = sum(log(N(w; mu, sigma))) + bias-term          (scaled 1/(B*R))

Strategy: data-parallel over batch B across 8 NeuronCores (4 batches each).
eps is uploaded in bf16 (halves HBM traffic; elementwise math runs in DVE
2x/4x bf16 modes).  Key algebraic restructurings (all exact up to tiny,
quantified approximations):

  log(mix(w)) with the reference's clamps is exactly
      max( softplus(ln10 - 49.5*u) + softplus(u/2 - 22.1069) - u/2 - ln(2*sqrt(2pi)),
           ln(1e-10) )                      with u = w^2
  The second softplus (active only for |w| in ~[5.7, 6.8]) is dropped and the
  outer max is folded into a min-clamp on t0 = u/2, so per element we need
  only:  u = w^2 ; t0 = min(u/2, 21.41377) ; s1 = ln(1 + exp(-99*t0 + ln10));
  lm = s1 - t0 - ln(2*sqrt(2pi)).  The constant is folded on the host.

  log(N(w; mu, sigma)) = clamp(-eps^2/2 - log(sqrt(2pi)*sigma), ln 1e-10, ln 10)
  whose sum is  16*sum(C) - sum(eps^2/2)  per core (clamp corrections are
  O(1e-5) relative and neglected; C = -log(sqrt(2pi)*sigma)).

  einsum('bi,brio->bro') = x @ (eps*sigma)  [bf16 on PE]  +  x @ mu  [fp32 on PE].

The bias path (tiny: 16x1024 per core) is computed with the full exact
formulas in fp32, including both softplus terms and both clamps.

Self-contained: hardcodes all shapes; only needs concourse (+numpy/ml_dtypes).
"""

import math
import os
import sys
from contextlib import ExitStack

import numpy as np

for _p in ("/opt/trn_rl_repo",):
    if _p not in sys.path:
        sys.path.insert(0, _p)

import ml_dtypes  # noqa: E402

import concourse.bacc as bacc  # noqa: E402
import concourse.bass as bass  # noqa: E402
import concourse.mybir as mybir  # noqa: E402
import concourse.tile as tile  # noqa: E402
from concourse.bass_utils import run_bass_kernel_spmd  # noqa: E402

# ----------------------------------------------------------------------------
# Problem constants (hardcoded per the harness contract)
N_CORES = 8
B, R, IN, OUT = 32, 4, 1024, 1024
BPC = B // N_CORES            # batches per core = 4
PLANES = BPC * R              # eps planes per core = 16
NCHUNK = IN // 128            # 8 partition chunks of the contraction dim
NTILES = PLANES * NCHUNK      # 128 main-loop tiles per core
NW_CORE = PLANES * IN * OUT   # eps elements per core

LN_SQRT_2PI = 0.5 * math.log(2.0 * math.pi)          # 0.918938533
K2 = math.log(2.0) + LN_SQRT_2PI                     # ln(2*sqrt(2pi)) = 1.61208571
LNC = math.log(1e-10)                                # -23.02585093
LN10 = math.log(10.0)                                # 2.30258509
CLAMP0 = -LNC - K2                                   # 21.41376521
SP2_BIAS = LNC + LN_SQRT_2PI                         # -22.10691240

F32 = mybir.dt.float32
BF16 = mybir.dt.bfloat16
AF = mybir.ActivationFunctionType
ALU = mybir.AluOpType

# All R=4 r-planes are processed together: tiles are [128, 4*OUT] to amortize
# the per-instruction fixed costs (ACT: 352 cyc, DVE: 58 cyc).
RPAIR = 4
NPAIRS = NTILES // RPAIR      # 32 quad tiles
# Every (H2_DVE_EVERY)-th quad computes sum(eps^2) as DVE-square + PE-reduce
# instead of ACT square+accumulate, to balance the engines.
H2_DVE_EVERY = 3

_CACHED = {}


def _pin_act_tables():
    """bacc's table chooser alternates Exp/Ln between two different table
    sets, paying a ~1.4us ACT_TABLE_LOAD per switch (240 loads!).  All the
    functions this kernel uses live together in natural_log_exp_and_others;
    strip them from every other set so the chooser has one stable answer."""
    if getattr(bacc, "_act_tables_pinned", False):
        return
    orig = bacc.get_activation_tables
    mine = {AF.Exp, AF.Ln, AF.Square, AF.Copy, AF.Identity}

    def patched(arch):
        tabs = orig(arch)
        return {
            name: (set(fns) if name == "natural_log_exp_and_others"
                   else set(fns) - mine)
            for name, fns in tabs.items()
        }

    bacc.get_activation_tables = patched
    bacc._act_tables_pinned = True


def _build_kernel():
    _pin_act_tables()
    nc = bacc.Bacc()

    eps_bf = nc.dram_tensor("eps_bf", [PLANES, IN, OUT], BF16, kind="ExternalInput")
    x_bf = nc.dram_tensor("x_bf", [128, NCHUNK * BPC], BF16, kind="ExternalInput")
    x_f32 = nc.dram_tensor("x_f32", [128, NCHUNK * BPC], F32, kind="ExternalInput")
    mu_d = nc.dram_tensor("mu", [IN, OUT], F32, kind="ExternalInput")
    ro_d = nc.dram_tensor("ro", [IN, OUT], F32, kind="ExternalInput")
    mu_b_d = nc.dram_tensor("mu_bias", [1, OUT], F32, kind="ExternalInput")
    ro_b_d = nc.dram_tensor("ro_bias", [1, OUT], F32, kind="ExternalInput")
    epsb_d = nc.dram_tensor("eps_bias", [PLANES, OUT], F32, kind="ExternalInput")
    blk_d = nc.dram_tensor("blk", [16, BPC], F32, kind="ExternalInput")

    out_d = nc.dram_tensor("out_shard", [BPC, OUT], F32, kind="ExternalOutput")
    part_d = nc.dram_tensor("partials", [1, 8], F32, kind="ExternalOutput")

    def bcast16(dram_h):
        ap0 = dram_h[:, :]
        return bass.AP(tensor=ap0.tensor, offset=ap0.offset, ap=[[0, 16], ap0.ap[-1]])

    with tile.TileContext(nc) as tc, ExitStack() as ctx:
        consts = ctx.enter_context(tc.tile_pool(name="consts", bufs=1))
        work = ctx.enter_context(tc.tile_pool(name="work", bufs=2))
        drain = ctx.enter_context(tc.tile_pool(name="drain", bufs=1))
        pre = ctx.enter_context(tc.tile_pool(name="pre", bufs=2))
        bias_p = ctx.enter_context(tc.tile_pool(name="biasp", bufs=1))
        ps_pre = ctx.enter_context(tc.tile_pool(name="ps_pre", bufs=1, space="PSUM"))
        ps_eins = ctx.enter_context(tc.tile_pool(name="ps_eins", bufs=1, space="PSUM"))
        ps_fin = ctx.enter_context(tc.tile_pool(name="ps_fin", bufs=1, space="PSUM"))

        # ------------------------- resident constants -------------------------
        sig_bf = consts.tile([128, NCHUNK, OUT], BF16)
        mu_bf = consts.tile([128, NCHUNK, OUT], BF16)
        x_bf_sb = consts.tile([128, NCHUNK * BPC], BF16)
        x_f32_sb = consts.tile([128, NCHUNK * BPC], F32)
        ones128 = consts.tile([128, 1], F32)
        blk_sb = consts.tile([16, BPC], F32)
        h2_slots = consts.tile([128, NPAIRS], F32)
        lnsig_slots = consts.tile([128, NCHUNK], F32)
        redbuf = consts.tile([128, 3], F32)
        redb16 = consts.tile([16, 2], F32)
        sum2_sb = consts.tile([BPC, OUT], F32)
        cb_ln10 = consts.tile([128, 1], F32)
        cb_sp2 = consts.tile([128, 1], F32)
        cb_a = consts.tile([128, 1], F32)

        A_K2 = math.exp(-K2)  # ln(A*x + A) = ln(x+1) - K2
        nc.vector.memset(cb_ln10, LN10)
        nc.vector.memset(cb_sp2, SP2_BIAS)
        nc.vector.memset(cb_a, A_K2)
        nc.vector.memset(ones128, 1.0)
        nc.vector.memset(h2_slots, 0.0)
        ones_bf = consts.tile([128, 1], BF16)
        nc.vector.memset(ones_bf, 1.0)
        nc.gpsimd.dma_start(out=x_bf_sb, in_=x_bf[:, :])
        nc.gpsimd.dma_start(out=x_f32_sb, in_=x_f32[:, :])
        nc.gpsimd.dma_start(out=blk_sb, in_=blk_d[:, :])

        # --------- preamble: sigma = softplus(ro) (bf16), mu cast, x@mu -------
        # sig_bf/mu_bf hold sigma/sqrt(2) and mu/sqrt(2) so that the squared
        # pre-activation w'^2 equals w^2/2 directly (saves a DVE op per tile).
        INV_SQRT2 = 0.70710678118654752
        for c in range(NCHUNK):
            ro_t = pre.tile([128, OUT], F32)
            nc.gpsimd.dma_start(out=ro_t, in_=ro_d[c * 128:(c + 1) * 128, :])
            nc.scalar.activation(ro_t, ro_t, AF.Exp)
            sig_f = pre.tile([128, OUT], F32)
            nc.scalar.activation(sig_f, ro_t, AF.Ln, bias=1.0)  # sigma fp32
            lnsig_scr = pre.tile([128, OUT], BF16)
            nc.scalar.activation(
                lnsig_scr, sig_f, AF.Ln,
                accum_out=lnsig_slots[:, c:c + 1],
            )
            nc.scalar.activation(sig_bf[:, c, :], sig_f, AF.Copy, scale=INV_SQRT2)
            mu_t = pre.tile([128, OUT], F32)
            nc.gpsimd.dma_start(out=mu_t, in_=mu_d[c * 128:(c + 1) * 128, :])
            nc.vector.tensor_scalar(
                out=mu_bf[:, c, :], in0=mu_t, scalar1=INV_SQRT2, scalar2=None,
                op0=ALU.mult,
            )

        # x@mu, one [4,512] PSUM half at a time (PSUM banks are scarce)
        xmub_sb = bias_p.tile([BPC, OUT], F32)
        for h in range(2):
            xmu_ps = ps_pre.tile([BPC, 512], F32, tag="xmu_ps")
            # dummy matmul: absorbs stray waits so real matmuls keep one wait
            nc.tensor.matmul(
                xmu_ps[0:BPC, 0:BPC], x_f32_sb[:, 0:BPC], x_f32_sb[:, 0:BPC],
                start=True, stop=True,
            )
            for c in range(NCHUNK):
                mu_t = pre.tile([128, 512], F32, tag="mu_t2")
                nc.gpsimd.dma_start(
                    out=mu_t,
                    in_=mu_d[c * 128:(c + 1) * 128, h * 512:(h + 1) * 512],
                )
                nc.tensor.matmul(
                    xmu_ps[0:BPC, :],
                    x_f32_sb[:, c * BPC:(c + 1) * BPC],
                    mu_t,
                    start=(c == 0),
                    stop=(c == NCHUNK - 1),
                )
            nc.scalar.activation(
                xmub_sb[:, h * 512:(h + 1) * 512], xmu_ps, AF.Copy
            )

        # ----------------------------- bias path ------------------------------
        # everything fp32, exact formulas (with both softplus terms and clamps);
        # tiles are aggressively reused in place (tiny path, off the hot loop).
        eb = bias_p.tile([16, OUT], F32)
        nc.gpsimd.dma_start(out=eb, in_=epsb_d[:, :])
        sig_b = bias_p.tile([16, OUT], F32)
        nc.gpsimd.dma_start(out=sig_b, in_=bcast16(ro_b_d))
        mu_b16 = bias_p.tile([16, OUT], F32)
        nc.gpsimd.dma_start(out=mu_b16, in_=bcast16(mu_b_d))

        nc.scalar.activation(sig_b, sig_b, AF.Exp)
        nc.scalar.activation(sig_b, sig_b, AF.Ln, bias=1.0)  # sigma_b
        c_b = bias_p.tile([16, OUT], F32)
        nc.scalar.activation(c_b, sig_b, AF.Ln)
        nc.vector.tensor_scalar(
            out=c_b, in0=c_b, scalar1=-1.0, scalar2=-LN_SQRT_2PI,
            op0=ALU.mult, op1=ALU.add,
        )
        # gaussian log-term: clamp(C_b - eb^2/2, LNC, LN10), exact, accumulated
        h2b = bias_p.tile([16, OUT], F32)
        nc.scalar.activation(h2b, eb, AF.Square, scale=0.70710678118654752)
        nc.vector.tensor_tensor(out=h2b, in0=c_b, in1=h2b, op=ALU.subtract)
        nc.vector.tensor_scalar(
            out=h2b, in0=h2b, scalar1=LN10, scalar2=LNC,
            op0=ALU.min, op1=ALU.max,
        )
        nc.vector.tensor_scalar(
            out=h2b, in0=h2b, scalar1=1.0, scalar2=0.0,
            op0=ALU.mult, op1=ALU.add, accum_out=redb16[:, 1:2],
        )
        # mixture log-term, exact with both softplus terms and final max-clamp
        w_b = bias_p.tile([16, OUT], F32)
        nc.vector.tensor_tensor(out=w_b, in0=eb, in1=sig_b, op=ALU.mult)
        nc.vector.tensor_tensor(out=w_b, in0=w_b, in1=mu_b16, op=ALU.add)
        u_b = bias_p.tile([16, OUT], F32)
        nc.vector.tensor_tensor(out=u_b, in0=w_b, in1=w_b, op=ALU.mult)
        t0u_b = bias_p.tile([16, OUT], F32)
        nc.vector.tensor_scalar(
            out=t0u_b, in0=u_b, scalar1=0.5, scalar2=None, op0=ALU.mult,
        )
        s1_b = bias_p.tile([16, OUT], F32)
        nc.scalar.activation(s1_b, t0u_b, AF.Exp, scale=-99.0, bias=cb_ln10[0:16, :])
        nc.scalar.activation(s1_b, s1_b, AF.Ln, bias=1.0)
        # u_b tile becomes t0s -> rho2 -> s2 in place
        nc.vector.tensor_scalar(
            out=u_b, in0=u_b, scalar1=0.5, scalar2=30.0,
            op0=ALU.mult, op1=ALU.min,
        )
        nc.scalar.activation(u_b, u_b, AF.Exp, scale=1.0, bias=cb_sp2[0:16, :])
        nc.scalar.activation(u_b, u_b, AF.Ln, bias=1.0)  # s2_b
        tmp_b = bias_p.tile([16, OUT], F32)
        nc.vector.scalar_tensor_tensor(
            out=tmp_b, in0=t0u_b, scalar=-1.0, in1=s1_b,
            op0=ALU.mult, op1=ALU.add,
        )
        nc.vector.tensor_tensor(out=tmp_b, in0=tmp_b, in1=u_b, op=ALU.add)
        nc.vector.tensor_scalar(
            out=tmp_b, in0=tmp_b, scalar1=-K2, scalar2=LNC,
            op0=ALU.add, op1=ALU.max,
        )
        nc.vector.tensor_scalar(
            out=tmp_b, in0=tmp_b, scalar1=1.0, scalar2=0.0,
            op0=ALU.mult, op1=ALU.add, accum_out=redb16[:, 0:1],
        )
        # per-b sums of bias values (for the output):  blk.T @ w_b
        # then sum2 = x@mu + 0.25 * bias_sums
        for h in range(2):
            bias_ps = ps_pre.tile([BPC, 512], F32, tag="xmu_ps")
            nc.tensor.matmul(
                bias_ps[0:BPC, :],
                blk_sb[:, 0:BPC],
                w_b[:, h * 512:(h + 1) * 512],
                start=True, stop=True,
            )
            nc.scalar.activation(
                sum2_sb[:, h * 512:(h + 1) * 512], bias_ps, AF.Copy, scale=0.25
            )
        nc.vector.tensor_tensor(out=sum2_sb, in0=xmub_sb, in1=sum2_sb, op=ALU.add)

        # ------------------------------ main loop ------------------------------
        # One pair of PSUM banks holds all four b-outputs: batch b accumulates
        # into partition row 32*b via PE column-group tiling.  No PSUM slot
        # reuse -> each group's first matmul carries only its single w1 wait.
        eins_lo = ps_eins.tile([128, 512], F32)
        eins_hi = ps_eins.tile([128, 512], F32)
        eins_h = [eins_lo, eins_hi]
        nc.tensor.matmul(
            eins_h[0][0:1, 0:4], x_bf_sb[:, 0:1], x_bf_sb[:, 0:4],
            start=True, stop=True,
        )
        # PSUM accumulators for the big log-sums: PE ones-matmul reductions.
        # (The DVE's fused tensor_scalar+accum runs at 1x -- far too slow.)
        lm_ps = ps_fin.tile([1, 512], F32)
        h2_ps = ps_fin.tile([1, 512], F32)

        def rep2(ap2d):
            # [128, N] operand viewed as [128, RPAIR, N] via stride-0 middle
            return bass.AP(
                tensor=ap2d.tensor, offset=ap2d.offset,
                ap=[ap2d.ap[0], [0, RPAIR], ap2d.ap[-1]],
            )

        dve_h2_ks = [k for k in range(NPAIRS)
                     if k % H2_DVE_EVERY == H2_DVE_EVERY - 1]
        for b in range(BPC):
            for c in range(NCHUNK):
                    k = b * NCHUNK + c
                    t_e = work.tile([128, RPAIR, OUT], BF16)
                    for ri in range(RPAIR):
                        plane = b * R + ri
                        nc.gpsimd.dma_start(
                            out=t_e[:, ri, :],
                            in_=eps_bf[plane, c * 128:(c + 1) * 128, :],
                        )
                    sig_c = rep2(sig_bf[:, c, :])
                    mu_c = rep2(mu_bf[:, c, :])
                    w1 = work.tile([128, RPAIR, OUT], BF16)
                    nc.vector.tensor_tensor(out=w1, in0=t_e, in1=sig_c, op=ALU.mult)
                    # wt holds w' = w/sqrt(2), then t0 = w'^2 = w^2/2 (in place)
                    wt = work.tile([128, RPAIR, OUT], BF16)
                    nc.vector.tensor_tensor(out=wt, in0=w1, in1=mu_c, op=ALU.add)
                    nc.vector.tensor_tensor(out=wt, in0=wt, in1=wt, op=ALU.mult)
                    # rho -> s1 = ln(1+rho) -> y = s1 - t0 -> clamp (in place).
                    # s1 is exactly 0 in bf16 for the (dominant) rho-underflow
                    # tail; the -K2 constant is folded out on the host:
                    # sum(max(y - K2, LNC)) = sum(max(y, LNC + K2)) - N*K2.
                    rho = work.tile([128, RPAIR, OUT], BF16)
                    nc.scalar.activation(rho, wt, AF.Exp, scale=-99.0, bias=cb_ln10)
                    nc.scalar.activation(rho, rho, AF.Ln, bias=1.0)
                    nc.vector.tensor_tensor(out=rho, in0=rho, in1=wt, op=ALU.subtract)
                    nc.vector.tensor_scalar(
                        out=rho, in0=rho, scalar1=-CLAMP0, scalar2=None,
                        op0=ALU.max,
                    )
                    # sum via PE: ones^T @ rho accumulated into one PSUM bank
                    for ri in range(RPAIR):
                        for h in range(2):
                            nc.tensor.matmul(
                                lm_ps[0:1, :], ones_bf[:, 0:1],
                                rho[:, ri, h * 512:(h + 1) * 512],
                                start=(k == 0 and ri == 0 and h == 0),
                                stop=(k == NPAIRS - 1 and ri == RPAIR - 1
                                      and h == 1),
                            )
                    if k in dve_h2_ks:
                        # eps^2 in place on t_e, then PE-reduce
                        nc.vector.tensor_tensor(
                            out=t_e, in0=t_e, in1=t_e, op=ALU.mult
                        )
                        for ri in range(RPAIR):
                            for h in range(2):
                                nc.tensor.matmul(
                                    h2_ps[0:1, :], ones_bf[:, 0:1],
                                    t_e[:, ri, h * 512:(h + 1) * 512],
                                    start=(k == dve_h2_ks[0] and ri == 0
                                           and h == 0),
                                    stop=(k == dve_h2_ks[-1] and ri == RPAIR - 1
                                          and h == 1),
                                )
                    else:
                        scr2 = work.tile([128, RPAIR, OUT], BF16)
                        nc.scalar.activation(
                            scr2, t_e, AF.Square, scale=1.0,
                            accum_out=h2_slots[:, k:k + 1],
                        )
                    for ri in range(RPAIR):
                        for h in range(2):
                            nc.tensor.matmul(
                                eins_h[h][32 * b:32 * b + 1, :],
                                x_bf_sb[:, c * BPC + b:c * BPC + b + 1],
                                w1[:, ri, h * 512:(h + 1) * 512],
                                start=(ri == 0 and c == 0),
                                stop=(ri == R - 1 and c == NCHUNK - 1),
                                tile_position=(0, 32 * b),
                            )

        # drain: out_row = sqrt(2)/4 * einsum + (x@mu + bias_sums/4)
        for b in range(BPC):
            e_sb = drain.tile([1, OUT], F32)
            for h in range(2):
                nc.vector.tensor_scalar(
                    out=e_sb[:, h * 512:(h + 1) * 512],
                    in0=eins_h[h][32 * b:32 * b + 1, :],
                    scalar1=0.35355339059327373, scalar2=None, op0=ALU.mult,
                )
            # engines can't address partition offsets 1-3: DMA the needed
            # sum2 row down to partition 0 first
            s2row = drain.tile([1, OUT], F32)
            nc.gpsimd.dma_start(out=s2row, in_=sum2_sb[b:b + 1, :])
            out_row = drain.tile([1, OUT], F32)
            nc.vector.tensor_tensor(
                out=out_row, in0=e_sb, in1=s2row, op=ALU.add
            )
            nc.gpsimd.dma_start(out=out_d[b:b + 1, :], in_=out_row)

        # ------------------------------- finale --------------------------------
        # partials layout: [0]=sum eps^2 (ACT part), [1]=sum ln sigma,
        # [2]=sum lm_bias, [3]=sum lg_bias, [4]=sum max(y,-C0) (lm raw),
        # [5]=sum eps^2 (DVE part)
        nc.vector.tensor_reduce(
            out=redbuf[:, 0:1], in_=h2_slots, axis=mybir.AxisListType.X, op=ALU.add
        )
        nc.vector.tensor_reduce(
            out=redbuf[:, 1:2], in_=lnsig_slots, axis=mybir.AxisListType.X, op=ALU.add
        )
        fin_ps = ps_fin.tile([1, 8], F32)
        nc.tensor.matmul(
            fin_ps[0:1, 0:2], ones128[:, 0:1], redbuf[:, 0:2], start=True, stop=True
        )
        nc.tensor.matmul(
            fin_ps[0:1, 2:4], ones128[0:16, 0:1], redb16[:, 0:2],
            start=True, stop=True,
        )
        fin_sb = consts.tile([1, 8], F32)
        nc.vector.memset(fin_sb, 0.0)
        nc.scalar.activation(fin_sb[0:1, 0:4], fin_ps[0:1, 0:4], AF.Copy)
        nc.vector.tensor_reduce(
            out=fin_sb[0:1, 4:5], in_=lm_ps[0:1, :],
            axis=mybir.AxisListType.X, op=ALU.add,
        )
        nc.vector.tensor_reduce(
            out=fin_sb[0:1, 5:6], in_=h2_ps[0:1, :],
            axis=mybir.AxisListType.X, op=ALU.add,
        )
        nc.gpsimd.dma_start(out=part_d[:, :], in_=fin_sb)

    nc.compile()
    return nc


def _prepare_in_maps(x, mu, ro, mu_bias, ro_bias, eps, eps_bias):
    x = np.asarray(x, np.float32)
    mu = np.asarray(mu, np.float32)
    ro = np.asarray(ro, np.float32)
    mu_bias = np.asarray(mu_bias, np.float32).reshape(1, OUT)
    ro_bias = np.asarray(ro_bias, np.float32).reshape(1, OUT)
    eps = np.asarray(eps)
    eps_bias = np.asarray(eps_bias, np.float32)

    blk = np.zeros((16, BPC), np.float32)
    for p in range(16):
        blk[p, p // R] = 1.0

    in_maps = []
    for core in range(N_CORES):
        b0 = core * BPC
        eps_sh = np.ascontiguousarray(eps[b0:b0 + BPC]).reshape(PLANES, IN, OUT)
        eps_sh = eps_sh.astype(ml_dtypes.bfloat16)
        x_sh = x[b0:b0 + BPC]                      # [BPC, IN]
        # x_cols[p, c*BPC + b] = x[b, c*128 + p]
        x_cols = np.ascontiguousarray(
            x_sh.reshape(BPC, NCHUNK, 128).transpose(2, 1, 0).reshape(128, NCHUNK * BPC)
        ).astype(np.float32)
        in_maps.append({
            "eps_bf": eps_sh,
            "x_bf": x_cols.astype(ml_dtypes.bfloat16),
            "x_f32": x_cols,
            "mu": mu,
            "ro": ro,
            "mu_bias": mu_bias,
            "ro_bias": ro_bias,
            "eps_bias": np.ascontiguousarray(
                eps_bias[b0:b0 + BPC]).reshape(PLANES, OUT),
            "blk": blk,
        })
    return in_maps


LAST_EXEC_NS = None


def kernel(x, mu, ro, mu_bias, ro_bias, eps, eps_bias):
    global LAST_EXEC_NS
    if "nc" not in _CACHED:
        _CACHED["nc"] = _build_kernel()
    nc = _CACHED["nc"]

    in_maps = _prepare_in_maps(x, mu, ro, mu_bias, ro_bias, eps, eps_bias)

    trace = bool(int(os.environ.get("KERNEL_TRACE", "0")))
    res = run_bass_kernel_spmd(
        nc, in_maps, list(range(N_CORES)), trace=trace,
    )
    LAST_EXEC_NS = res.exec_time_ns
    results = res.results

    out = np.concatenate(
        [np.asarray(results[c]["out_shard"], np.float64) for c in range(N_CORES)],
        axis=0,
    )

    denom = float(B * R)
    lp_sum = 0.0
    lpw_sum = 0.0
    for c in range(N_CORES):
        p = np.asarray(results[c]["partials"], np.float64).reshape(-1)
        sum_eps2 = p[0] + p[5]
        lnsig, lm_b, lg_b = p[1], p[2], p[3]
        lm_w = p[4] - NW_CORE * K2
        sum_c = -(IN * OUT) * LN_SQRT_2PI - lnsig
        lg_w = PLANES * sum_c - 0.5 * sum_eps2
        lp_sum += lm_w + lm_b
        lpw_sum += lg_w + lg_b
    log_prior = np.float32(lp_sum / denom)
    log_p = np.float32(lpw_sum / denom)

    return out.astype(np.float32), log_prior, log_p


# revision 72
# speedup vs baseline: 1.4332x; 1.0307x over previous
"""Trainium2 Bass kernel for the BayesianLayer problem.

Computes, for
    sigma   = softplus(ro)                      (IN, OUT)
    weights = eps * sigma + mu                  (B, R, IN, OUT)
    bias    = eps_bias * softplus(ro_bias) + mu_bias
    out     = mean_r(x @ weights + bias)        (B, OUT)
    log_prior = sum(log(mix(weights))) + sum(log(mix(bias)))   (scaled 1/(B*R))
    log_p     = sum(log(N(w; mu, sigma))) + bias-term          (scaled 1/(B*R))

Strategy: data-parallel over batch B across 8 NeuronCores (4 batches each).
eps is uploaded in bf16 (halves HBM traffic; elementwise math runs in DVE
2x/4x bf16 modes).  Key algebraic restructurings (all exact up to tiny,
quantified approximations):

  log(mix(w)) with the reference's clamps is exactly
      max( softplus(ln10 - 49.5*u) + softplus(u/2 - 22.1069) - u/2 - ln(2*sqrt(2pi)),
           ln(1e-10) )                      with u = w^2
  The second softplus (active only for |w| in ~[5.7, 6.8]) is dropped and the
  outer max is folded into a min-clamp on t0 = u/2, so per element we need
  only:  u = w^2 ; t0 = min(u/2, 21.41377) ; s1 = ln(1 + exp(-99*t0 + ln10));
  lm = s1 - t0 - ln(2*sqrt(2pi)).  The constant is folded on the host.

  log(N(w; mu, sigma)) = clamp(-eps^2/2 - log(sqrt(2pi)*sigma), ln 1e-10, ln 10)
  whose sum is  16*sum(C) - sum(eps^2/2)  per core (clamp corrections are
  O(1e-5) relative and neglected; C = -log(sqrt(2pi)*sigma)).

  einsum('bi,brio->bro') = x @ (eps*sigma)  [bf16 on PE]  +  x @ mu  [fp32 on PE].

The bias path (tiny: 16x1024 per core) is computed with the full exact
formulas in fp32, including both softplus terms and both clamps.

Self-contained: hardcodes all shapes; only needs concourse (+numpy/ml_dtypes).
"""

import math
import os
import sys
from contextlib import ExitStack

import numpy as np

for _p in ("/opt/trn_rl_repo",):
    if _p not in sys.path:
        sys.path.insert(0, _p)

import ml_dtypes  # noqa: E402

import concourse.bacc as bacc  # noqa: E402
import concourse.bass as bass  # noqa: E402
import concourse.mybir as mybir  # noqa: E402
import concourse.tile as tile  # noqa: E402
from concourse.bass_utils import run_bass_kernel_spmd  # noqa: E402

# ----------------------------------------------------------------------------
# Problem constants (hardcoded per the harness contract)
N_CORES = 8
B, R, IN, OUT = 32, 4, 1024, 1024
BPC = B // N_CORES            # batches per core = 4
PLANES = BPC * R              # eps planes per core = 16
NCHUNK = IN // 128            # 8 partition chunks of the contraction dim
NTILES = PLANES * NCHUNK      # 128 main-loop tiles per core
NW_CORE = PLANES * IN * OUT   # eps elements per core

LN_SQRT_2PI = 0.5 * math.log(2.0 * math.pi)          # 0.918938533
K2 = math.log(2.0) + LN_SQRT_2PI                     # ln(2*sqrt(2pi)) = 1.61208571
LNC = math.log(1e-10)                                # -23.02585093
LN10 = math.log(10.0)                                # 2.30258509
CLAMP0 = -LNC - K2                                   # 21.41376521
SP2_BIAS = LNC + LN_SQRT_2PI                         # -22.10691240

F32 = mybir.dt.float32
BF16 = mybir.dt.bfloat16
AF = mybir.ActivationFunctionType
ALU = mybir.AluOpType

# All R=4 r-planes are processed together: tiles are [128, 4*OUT] to amortize
# the per-instruction fixed costs (ACT: 352 cyc, DVE: 58 cyc).
RPAIR = 4
NPAIRS = NTILES // RPAIR      # 32 quad tiles
# Every (H2_DVE_EVERY)-th quad computes sum(eps^2) as DVE-square + PE-reduce
# instead of ACT square+accumulate, to balance the engines.
H2_DVE_EVERY = 3

_CACHED = {}


def _pin_act_tables():
    """bacc's table chooser alternates Exp/Ln between two different table
    sets, paying a ~1.4us ACT_TABLE_LOAD per switch (240 loads!).  All the
    functions this kernel uses live together in natural_log_exp_and_others;
    strip them from every other set so the chooser has one stable answer."""
    if getattr(bacc, "_act_tables_pinned", False):
        return
    orig = bacc.get_activation_tables
    mine = {AF.Exp, AF.Ln, AF.Square, AF.Copy, AF.Identity}

    def patched(arch):
        tabs = orig(arch)
        return {
            name: (set(fns) if name == "natural_log_exp_and_others"
                   else set(fns) - mine)
            for name, fns in tabs.items()
        }

    bacc.get_activation_tables = patched
    bacc._act_tables_pinned = True


def _build_kernel():
    _pin_act_tables()
    nc = bacc.Bacc()

    eps_bf = nc.dram_tensor("eps_bf", [PLANES, IN, OUT], BF16, kind="ExternalInput")
    x_bf = nc.dram_tensor("x_bf", [128, NCHUNK * BPC], BF16, kind="ExternalInput")
    x_f32 = nc.dram_tensor("x_f32", [128, NCHUNK * BPC], F32, kind="ExternalInput")
    mu_d = nc.dram_tensor("mu", [IN, OUT], F32, kind="ExternalInput")
    ro_d = nc.dram_tensor("ro", [IN, OUT], F32, kind="ExternalInput")
    mu_b_d = nc.dram_tensor("mu_bias", [1, OUT], F32, kind="ExternalInput")
    ro_b_d = nc.dram_tensor("ro_bias", [1, OUT], F32, kind="ExternalInput")
    epsb_d = nc.dram_tensor("eps_bias", [PLANES, OUT], F32, kind="ExternalInput")
    blk_d = nc.dram_tensor("blk", [16, BPC], F32, kind="ExternalInput")

    out_d = nc.dram_tensor("out_shard", [BPC, OUT], F32, kind="ExternalOutput")
    part_d = nc.dram_tensor("partials", [1, 8], F32, kind="ExternalOutput")

    def bcast16(dram_h):
        ap0 = dram_h[:, :]
        return bass.AP(tensor=ap0.tensor, offset=ap0.offset, ap=[[0, 16], ap0.ap[-1]])

    with tile.TileContext(nc) as tc, ExitStack() as ctx:
        consts = ctx.enter_context(tc.tile_pool(name="consts", bufs=1))
        work = ctx.enter_context(tc.tile_pool(name="work", bufs=3))
        drain = ctx.enter_context(tc.tile_pool(name="drain", bufs=1))
        pre = ctx.enter_context(tc.tile_pool(name="pre", bufs=1))
        bias_p = ctx.enter_context(tc.tile_pool(name="biasp", bufs=1))
        ps_pre = ctx.enter_context(tc.tile_pool(name="ps_pre", bufs=1, space="PSUM"))
        ps_eins = ctx.enter_context(tc.tile_pool(name="ps_eins", bufs=1, space="PSUM"))
        ps_fin = ctx.enter_context(tc.tile_pool(name="ps_fin", bufs=1, space="PSUM"))

        # ------------------------- resident constants -------------------------
        sig_bf = consts.tile([128, NCHUNK, OUT], BF16)
        mu_bf = consts.tile([128, NCHUNK, OUT], BF16)
        x_bf_sb = consts.tile([128, NCHUNK * BPC], BF16)
        x_f32_sb = consts.tile([128, NCHUNK * BPC], F32)
        ones128 = consts.tile([128, 1], F32)
        blk_sb = consts.tile([16, BPC], F32)
        h2_slots = consts.tile([128, NPAIRS], F32)
        lnsig_slots = consts.tile([128, NCHUNK], F32)
        redbuf = consts.tile([128, 3], F32)
        redb16 = consts.tile([16, 4], F32)
        sum2_sb = consts.tile([BPC, OUT], F32)
        cb_ln10 = consts.tile([128, 1], F32)
        cb_sp2 = consts.tile([128, 1], F32)
        cb_a = consts.tile([128, 1], F32)

        A_K2 = math.exp(-K2)  # ln(A*x + A) = ln(x+1) - K2
        nc.vector.memset(cb_ln10, LN10)
        nc.vector.memset(cb_sp2, SP2_BIAS)
        nc.vector.memset(cb_a, A_K2)
        nc.vector.memset(ones128, 1.0)
        nc.vector.memset(h2_slots, 0.0)
        ones_bf = consts.tile([128, 1], BF16)
        nc.vector.memset(ones_bf, 1.0)
        nc.gpsimd.dma_start(out=x_bf_sb, in_=x_bf[:, :])
        nc.gpsimd.dma_start(out=x_f32_sb, in_=x_f32[:, :])
        nc.gpsimd.dma_start(out=blk_sb, in_=blk_d[:, :])

        # --------- preamble: sigma = softplus(ro) (bf16), mu cast, x@mu -------
        # sig_bf/mu_bf hold sigma/sqrt(2) and mu/sqrt(2) so that the squared
        # pre-activation w'^2 equals w^2/2 directly (saves a DVE op per tile).
        INV_SQRT2 = 0.70710678118654752
        for c in range(NCHUNK):
            ro_t = pre.tile([128, OUT], F32)
            nc.gpsimd.dma_start(out=ro_t, in_=ro_d[c * 128:(c + 1) * 128, :])
            nc.scalar.activation(ro_t, ro_t, AF.Exp)
            sig_f = pre.tile([128, OUT], F32)
            nc.scalar.activation(sig_f, ro_t, AF.Ln, bias=1.0)  # sigma fp32
            lnsig_scr = pre.tile([128, OUT], BF16)
            nc.scalar.activation(
                lnsig_scr, sig_f, AF.Ln,
                accum_out=lnsig_slots[:, c:c + 1],
            )
            nc.scalar.activation(sig_bf[:, c, :], sig_f, AF.Copy, scale=INV_SQRT2)
            mu_t = pre.tile([128, OUT], F32)
            nc.gpsimd.dma_start(out=mu_t, in_=mu_d[c * 128:(c + 1) * 128, :])
            nc.vector.tensor_scalar(
                out=mu_bf[:, c, :], in0=mu_t, scalar1=INV_SQRT2, scalar2=None,
                op0=ALU.mult,
            )

        # x@mu, one [4,512] PSUM half at a time (PSUM banks are scarce)
        xmub_sb = bias_p.tile([BPC, OUT], F32)
        for h in range(2):
            xmu_ps = ps_pre.tile([BPC, 512], F32, tag="xmu_ps")
            # dummy matmul: absorbs stray waits so real matmuls keep one wait
            nc.tensor.matmul(
                xmu_ps[0:BPC, 0:BPC], x_f32_sb[:, 0:BPC], x_f32_sb[:, 0:BPC],
                start=True, stop=True,
            )
            for c in range(NCHUNK):
                mu_t = pre.tile([128, 512], F32, tag="mu_t2")
                nc.gpsimd.dma_start(
                    out=mu_t,
                    in_=mu_d[c * 128:(c + 1) * 128, h * 512:(h + 1) * 512],
                )
                nc.tensor.matmul(
                    xmu_ps[0:BPC, :],
                    x_f32_sb[:, c * BPC:(c + 1) * BPC],
                    mu_t,
                    start=(c == 0),
                    stop=(c == NCHUNK - 1),
                )
            nc.scalar.activation(
                xmub_sb[:, h * 512:(h + 1) * 512], xmu_ps, AF.Copy
            )

        # ----------------------------- bias path ------------------------------
        # everything fp32, exact formulas (with both softplus terms and clamps);
        # tiles reused in place and processed in two 512-column halves to save
        # SBUF (tiny path, off the hot loop).
        sig_b = bias_p.tile([16, OUT], F32)
        nc.gpsimd.dma_start(out=sig_b, in_=bcast16(ro_b_d))
        nc.scalar.activation(sig_b, sig_b, AF.Exp)
        nc.scalar.activation(sig_b, sig_b, AF.Ln, bias=1.0)  # sigma_b

        H = 512
        for h in range(2):
            sl = slice(h * H, (h + 1) * H)
            eb = bias_p.tile([16, H], F32)
            nc.gpsimd.dma_start(out=eb, in_=epsb_d[:, sl])
            mu_b16 = bias_p.tile([16, H], F32)
            mb_ap = bcast16(mu_b_d)
            mu_bh = bass.AP(
                tensor=mb_ap.tensor, offset=mb_ap.offset + h * H,
                ap=[[0, 16], [1, H]],
            )
            nc.gpsimd.dma_start(out=mu_b16, in_=mu_bh)
            c_b = bias_p.tile([16, H], F32)
            nc.scalar.activation(c_b, sig_b[:, sl], AF.Ln)
            nc.vector.tensor_scalar(
                out=c_b, in0=c_b, scalar1=-1.0, scalar2=-LN_SQRT_2PI,
                op0=ALU.mult, op1=ALU.add,
            )
            # gaussian log-term: clamp(C_b - eb^2/2, LNC, LN10), exact
            h2b = bias_p.tile([16, H], F32)
            nc.scalar.activation(h2b, eb, AF.Square, scale=0.70710678118654752)
            nc.vector.tensor_tensor(out=h2b, in0=c_b, in1=h2b, op=ALU.subtract)
            nc.vector.tensor_scalar(
                out=h2b, in0=h2b, scalar1=LN10, scalar2=LNC,
                op0=ALU.min, op1=ALU.max,
            )
            nc.vector.tensor_scalar(
                out=h2b, in0=h2b, scalar1=1.0, scalar2=0.0,
                op0=ALU.mult, op1=ALU.add, accum_out=redb16[:, 2 + h:3 + h],
            )
            # mixture log-term, exact (both softplus terms, final max-clamp)
            w_b = bias_p.tile([16, H], F32)
            nc.vector.tensor_tensor(out=w_b, in0=eb, in1=sig_b[:, sl], op=ALU.mult)
            nc.vector.tensor_tensor(out=w_b, in0=w_b, in1=mu_b16, op=ALU.add)
            u_b = bias_p.tile([16, H], F32)
            nc.vector.tensor_tensor(out=u_b, in0=w_b, in1=w_b, op=ALU.mult)
            t0u_b = bias_p.tile([16, H], F32)
            nc.vector.tensor_scalar(
                out=t0u_b, in0=u_b, scalar1=0.5, scalar2=None, op0=ALU.mult,
            )
            s1_b = bias_p.tile([16, H], F32)
            nc.scalar.activation(
                s1_b, t0u_b, AF.Exp, scale=-99.0, bias=cb_ln10[0:16, :]
            )
            nc.scalar.activation(s1_b, s1_b, AF.Ln, bias=1.0)
            # u_b tile becomes t0s -> rho2 -> s2 in place
            nc.vector.tensor_scalar(
                out=u_b, in0=u_b, scalar1=0.5, scalar2=30.0,
                op0=ALU.mult, op1=ALU.min,
            )
            nc.scalar.activation(u_b, u_b, AF.Exp, scale=1.0, bias=cb_sp2[0:16, :])
            nc.scalar.activation(u_b, u_b, AF.Ln, bias=1.0)  # s2_b
            tmp_b = bias_p.tile([16, H], F32)
            nc.vector.scalar_tensor_tensor(
                out=tmp_b, in0=t0u_b, scalar=-1.0, in1=s1_b,
                op0=ALU.mult, op1=ALU.add,
            )
            nc.vector.tensor_tensor(out=tmp_b, in0=tmp_b, in1=u_b, op=ALU.add)
            nc.vector.tensor_scalar(
                out=tmp_b, in0=tmp_b, scalar1=-K2, scalar2=LNC,
                op0=ALU.add, op1=ALU.max,
            )
            nc.vector.tensor_scalar(
                out=tmp_b, in0=tmp_b, scalar1=1.0, scalar2=0.0,
                op0=ALU.mult, op1=ALU.add, accum_out=redb16[:, h:h + 1],
            )
            # per-b sums of bias values, then sum2 = x@mu + 0.25 * bias_sums
            bias_ps = ps_pre.tile([BPC, 512], F32, tag="xmu_ps")
            nc.tensor.matmul(
                bias_ps[0:BPC, :], blk_sb[:, 0:BPC], w_b,
                start=True, stop=True,
            )
            nc.scalar.activation(sum2_sb[:, sl], bias_ps, AF.Copy, scale=0.25)
        nc.vector.tensor_tensor(out=sum2_sb, in0=xmub_sb, in1=sum2_sb, op=ALU.add)

        # ------------------------------ main loop ------------------------------
        # One pair of PSUM banks holds all four b-outputs: batch b accumulates
        # into partition row 32*b via PE column-group tiling.  No PSUM slot
        # reuse -> each group's first matmul carries only its single w1 wait.
        eins_lo = ps_eins.tile([128, 512], F32)
        eins_hi = ps_eins.tile([128, 512], F32)
        eins_h = [eins_lo, eins_hi]
        nc.tensor.matmul(
            eins_h[0][0:1, 0:4], x_bf_sb[:, 0:1], x_bf_sb[:, 0:4],
            start=True, stop=True,
        )
        # PSUM accumulators for the big log-sums: PE ones-matmul reductions.
        # (The DVE's fused tensor_scalar+accum runs at 1x -- far too slow.)
        lm_ps = ps_fin.tile([1, 512], F32)
        h2_ps = ps_fin.tile([1, 512], F32)

        def rep2(ap2d):
            # [128, N] operand viewed as [128, RPAIR, N] via stride-0 middle
            return bass.AP(
                tensor=ap2d.tensor, offset=ap2d.offset,
                ap=[ap2d.ap[0], [0, RPAIR], ap2d.ap[-1]],
            )

        dve_h2_ks = [k for k in range(NPAIRS)
                     if k % H2_DVE_EVERY == H2_DVE_EVERY - 1]
        for b in range(BPC):
            for c in range(NCHUNK):
                    k = b * NCHUNK + c
                    t_e = work.tile([128, RPAIR, OUT], BF16)
                    for ri in range(RPAIR):
                        plane = b * R + ri
                        nc.gpsimd.dma_start(
                            out=t_e[:, ri, :],
                            in_=eps_bf[plane, c * 128:(c + 1) * 128, :],
                        )
                    sig_c = rep2(sig_bf[:, c, :])
                    mu_c = rep2(mu_bf[:, c, :])
                    w1 = work.tile([128, RPAIR, OUT], BF16)
                    nc.vector.tensor_tensor(out=w1, in0=t_e, in1=sig_c, op=ALU.mult)
                    # wt holds w' = w/sqrt(2), then t0 = w'^2 = w^2/2 (in place)
                    wt = work.tile([128, RPAIR, OUT], BF16)
                    nc.vector.tensor_tensor(out=wt, in0=w1, in1=mu_c, op=ALU.add)
                    nc.vector.tensor_tensor(out=wt, in0=wt, in1=wt, op=ALU.mult)
                    # rho -> s1 = ln(1+rho) -> y = s1 - t0 -> clamp (in place).
                    # s1 is exactly 0 in bf16 for the (dominant) rho-underflow
                    # tail; the -K2 constant is folded out on the host:
                    # sum(max(y - K2, LNC)) = sum(max(y, LNC + K2)) - N*K2.
                    rho = work.tile([128, RPAIR, OUT], BF16)
                    nc.scalar.activation(rho, wt, AF.Exp, scale=-99.0, bias=cb_ln10)
                    nc.scalar.activation(rho, rho, AF.Ln, bias=1.0)
                    nc.vector.tensor_tensor(out=rho, in0=rho, in1=wt, op=ALU.subtract)
                    nc.vector.tensor_scalar(
                        out=rho, in0=rho, scalar1=-CLAMP0, scalar2=None,
                        op0=ALU.max,
                    )
                    # sum via PE: ones^T @ rho accumulated into one PSUM bank
                    for ri in range(RPAIR):
                        for h in range(2):
                            nc.tensor.matmul(
                                lm_ps[0:1, :], ones_bf[:, 0:1],
                                rho[:, ri, h * 512:(h + 1) * 512],
                                start=(k == 0 and ri == 0 and h == 0),
                                stop=(k == NPAIRS - 1 and ri == RPAIR - 1
                                      and h == 1),
                            )
                    if k in dve_h2_ks:
                        # eps^2 in place on t_e, then PE-reduce
                        nc.vector.tensor_tensor(
                            out=t_e, in0=t_e, in1=t_e, op=ALU.mult
                        )
                        for ri in range(RPAIR):
                            for h in range(2):
                                nc.tensor.matmul(
                                    h2_ps[0:1, :], ones_bf[:, 0:1],
                                    t_e[:, ri, h * 512:(h + 1) * 512],
                                    start=(k == dve_h2_ks[0] and ri == 0
                                           and h == 0),
                                    stop=(k == dve_h2_ks[-1] and ri == RPAIR - 1
                                          and h == 1),
                                )
                    else:
                        scr2 = work.tile([128, RPAIR, OUT], BF16)
                        nc.scalar.activation(
                            scr2, t_e, AF.Square, scale=1.0,
                            accum_out=h2_slots[:, k:k + 1],
                        )
                    for ri in range(RPAIR):
                        for h in range(2):
                            nc.tensor.matmul(
                                eins_h[h][32 * b:32 * b + 1, :],
                                x_bf_sb[:, c * BPC + b:c * BPC + b + 1],
                                w1[:, ri, h * 512:(h + 1) * 512],
                                start=(ri == 0 and c == 0),
                                stop=(ri == R - 1 and c == NCHUNK - 1),
                                tile_position=(0, 32 * b),
                            )

        # drain: out_row = sqrt(2)/4 * einsum + (x@mu + bias_sums/4)
        for b in range(BPC):
            e_sb = drain.tile([1, OUT], F32)
            for h in range(2):
                nc.vector.tensor_scalar(
                    out=e_sb[:, h * 512:(h + 1) * 512],
                    in0=eins_h[h][32 * b:32 * b + 1, :],
                    scalar1=0.35355339059327373, scalar2=None, op0=ALU.mult,
                )
            # engines can't address partition offsets 1-3: DMA the needed
            # sum2 row down to partition 0 first
            s2row = drain.tile([1, OUT], F32)
            nc.gpsimd.dma_start(out=s2row, in_=sum2_sb[b:b + 1, :])
            nc.vector.tensor_tensor(
                out=e_sb, in0=e_sb, in1=s2row, op=ALU.add
            )
            nc.gpsimd.dma_start(out=out_d[b:b + 1, :], in_=e_sb)

        # ------------------------------- finale --------------------------------
        # partials layout: [0]=sum eps^2 (ACT part), [1]=sum ln sigma,
        # [2],[3]=sum lm_bias halves, [4],[5]=sum lg_bias halves,
        # [6]=sum max(y,-C0) (lm raw), [7]=sum eps^2 (DVE part)
        nc.vector.tensor_reduce(
            out=redbuf[:, 0:1], in_=h2_slots, axis=mybir.AxisListType.X, op=ALU.add
        )
        nc.vector.tensor_reduce(
            out=redbuf[:, 1:2], in_=lnsig_slots, axis=mybir.AxisListType.X, op=ALU.add
        )
        fin_ps = ps_fin.tile([1, 8], F32)
        nc.tensor.matmul(
            fin_ps[0:1, 0:2], ones128[:, 0:1], redbuf[:, 0:2], start=True, stop=True
        )
        nc.tensor.matmul(
            fin_ps[0:1, 2:6], ones128[0:16, 0:1], redb16[:, 0:4],
            start=True, stop=True,
        )
        fin_sb = consts.tile([1, 8], F32)
        nc.vector.memset(fin_sb, 0.0)
        nc.scalar.activation(fin_sb[0:1, 0:6], fin_ps[0:1, 0:6], AF.Copy)
        nc.vector.tensor_reduce(
            out=fin_sb[0:1, 6:7], in_=lm_ps[0:1, :],
            axis=mybir.AxisListType.X, op=ALU.add,
        )
        nc.vector.tensor_reduce(
            out=fin_sb[0:1, 7:8], in_=h2_ps[0:1, :],
            axis=mybir.AxisListType.X, op=ALU.add,
        )
        nc.gpsimd.dma_start(out=part_d[:, :], in_=fin_sb)

    nc.compile()
    return nc


def _prepare_in_maps(x, mu, ro, mu_bias, ro_bias, eps, eps_bias):
    x = np.asarray(x, np.float32)
    mu = np.asarray(mu, np.float32)
    ro = np.asarray(ro, np.float32)
    mu_bias = np.asarray(mu_bias, np.float32).reshape(1, OUT)
    ro_bias = np.asarray(ro_bias, np.float32).reshape(1, OUT)
    eps = np.asarray(eps)
    eps_bias = np.asarray(eps_bias, np.float32)

    blk = np.zeros((16, BPC), np.float32)
    for p in range(16):
        blk[p, p // R] = 1.0

    in_maps = []
    for core in range(N_CORES):
        b0 = core * BPC
        eps_sh = np.ascontiguousarray(eps[b0:b0 + BPC]).reshape(PLANES, IN, OUT)
        eps_sh = eps_sh.astype(ml_dtypes.bfloat16)
        x_sh = x[b0:b0 + BPC]                      # [BPC, IN]
        # x_cols[p, c*BPC + b] = x[b, c*128 + p]
        x_cols = np.ascontiguousarray(
            x_sh.reshape(BPC, NCHUNK, 128).transpose(2, 1, 0).reshape(128, NCHUNK * BPC)
        ).astype(np.float32)
        in_maps.append({
            "eps_bf": eps_sh,
            "x_bf": x_cols.astype(ml_dtypes.bfloat16),
            "x_f32": x_cols,
            "mu": mu,
            "ro": ro,
            "mu_bias": mu_bias,
            "ro_bias": ro_bias,
            "eps_bias": np.ascontiguousarray(
                eps_bias[b0:b0 + BPC]).reshape(PLANES, OUT),
            "blk": blk,
        })
    return in_maps


LAST_EXEC_NS = None


def kernel(x, mu, ro, mu_bias, ro_bias, eps, eps_bias):
    global LAST_EXEC_NS
    if "nc" not in _CACHED:
        _CACHED["nc"] = _build_kernel()
    nc = _CACHED["nc"]

    in_maps = _prepare_in_maps(x, mu, ro, mu_bias, ro_bias, eps, eps_bias)

    trace = bool(int(os.environ.get("KERNEL_TRACE", "0")))
    res = run_bass_kernel_spmd(
        nc, in_maps, list(range(N_CORES)), trace=trace,
    )
    LAST_EXEC_NS = res.exec_time_ns
    results = res.results

    out = np.concatenate(
        [np.asarray(results[c]["out_shard"], np.float64) for c in range(N_CORES)],
        axis=0,
    )

    denom = float(B * R)
    lp_sum = 0.0
    lpw_sum = 0.0
    for c in range(N_CORES):
        p = np.asarray(results[c]["partials"], np.float64).reshape(-1)
        sum_eps2 = p[0] + p[7]
        lnsig = p[1]
        lm_b = p[2] + p[3]
        lg_b = p[4] + p[5]
        lm_w = p[6] - NW_CORE * K2
        sum_c = -(IN * OUT) * LN_SQRT_2PI - lnsig
        lg_w = PLANES * sum_c - 0.5 * sum_eps2
        lp_sum += lm_w + lm_b
        lpw_sum += lg_w + lg_b
    log_prior = np.float32(lp_sum / denom)
    log_p = np.float32(lpw_sum / denom)

    return out.astype(np.float32), log_prior, log_p
